# Initial kernel scaffold
#
"""Trainium2 Bass kernel for nn_Base2DInference (sampling).

Data-parallel over the sample batch B across 8 NeuronCores. Per core:
  - tiny MLP 10->32->32->32->32 in fp32 on the PE, 4 sample-groups packed
    into the 128x128 array (K,M=32 blocks)
  - PE transpose of the MLP head to samples-on-partitions layout
  - index math (rotation, texel indices, z bin) on DVE/ACT
  - per-bin norm factors via an is_ge staircase select
  - one big indirect-DMA gather per supertile from an (pdf,fac)-interleaved
    replicated DRAM table (8B per sample-component)
  - weighted mixture reduction -> [B/8] output
"""
import sys, os, time, types

sys.path.insert(0, '/opt/trn_rl_repo')

import numpy as np


def _install_ntff_hook_shim():
    if 'antenv.axon_hooks' in sys.modules:
        return
    try:
        from trn_agent_boot.trn_boot import _ntff_profile_via_ctypes
        hook = _ntff_profile_via_ctypes('/opt/axon/libaxon_pjrt.so')
    except Exception:
        hook = None
    mod = types.ModuleType('antenv.axon_hooks')
    _state = {'hook': hook}
    mod.set_axon_ntff_profile_hook = lambda h: _state.__setitem__('hook', h)
    mod.get_axon_ntff_profile_hook = lambda: _state['hook']
    sys.modules['antenv.axon_hooks'] = mod


_install_ntff_hook_shim()

from concourse import bass, mybir, bacc, tile
from concourse.bass_utils import run_bass_kernel_spmd

F32 = mybir.dt.float32
BF16 = mybir.dt.bfloat16
I32 = mybir.dt.int32

RES, ANG, D, HID, CIN = 512, 8, 8, 32, 10
B = 1048576
NC_N = 8
BC = B // NC_N            # samples per core: 131072
NQ = 4                    # packed sample groups per matmul
QS = BC // NQ             # samples per group: 32768
NT = QS // 512            # super-tiles: 64
NCHUNK = BC // 128        # 128-sample chunks: 1024
NW = NCHUNK * D           # texel columns: 8192

_MAXW = 1


def _patched_drain_and_barrier(self, tick_clock, wait_clock):
    from concourse.vector_clock import ScopedClock
    nc = self.nc
    drain_inst = nc.sync.drain()
    wait_clock.add_sem_waits(drain_inst.ins, ScopedClock({None: tick_clock.global_clock}))
    si = drain_inst.ins.sync_info
    if si is not None and si.on_wait and len(si.on_wait) > _MAXW:
        waits = list(si.on_wait)
        drain_inst.ins.sync_info = mybir.SyncInfo(
            on_wait=waits[:_MAXW], on_update=list(si.on_update))
        for i in range(_MAXW, len(waits), _MAXW):
            nop = nc.sync.nop(nofuse=True)
            nop.ins.sync_info = mybir.SyncInfo(on_wait=waits[i:i + _MAXW], on_update=[])
    nc.all_engine_barrier()
    popped = nc._tile_sem_poison_stack.pop()
    assert popped is self._sem_poison
    nc.clear_and_free_semaphores(list(self.sems.allocated().values()))
    nc.all_engine_barrier()


tile.TileContext._drain_and_barrier = _patched_drain_and_barrier


def split_sync_waits(nc, max_waits=_MAXW):
    fn = nc.m.functions[0]
    root_bb = nc.cur_bb.bb
    for bb in list(fn.blocks):
        insts = bb.instructions
        changed = False
        out = []
        for inst in insts:
            si = inst.sync_info
            waits = list(si.on_wait) if (si is not None and si.on_wait) else []
            if len(waits) > max_waits:
                changed = True
                extra = waits[:-max_waits]
                inst.sync_info = mybir.SyncInfo(
                    on_wait=waits[-max_waits:], on_update=list(si.on_update))
                for j in range(0, len(extra), max_waits):
                    nop = nc.engines[inst.engine].nop(nofuse=True)
                    nop.ins.sync_info = mybir.SyncInfo(
                        on_wait=extra[j:j + max_waits], on_update=[])
                    root_insts = root_bb.instructions
                    assert root_insts[-1].name == nop.ins.name
                    root_bb.instructions = root_insts[:-1]
                    out.append(nop.ins)
            out.append(inst)
        if changed:
            bb.instructions = out


class PatchedBacc(bacc.Bacc):
    def finalize(self):
        self.compile()
        split_sync_waits(self)
        self.verify_switch_hints()
        self.assert_all_executable()
        self.freeze()
        self._finalized = True


def build_kernel():
    AL = mybir.AluOpType
    AF = mybir.ActivationFunctionType
    nc = PatchedBacc()

    cond_t = nc.declare_dram_parameter("cond_t", [NQ * CIN, QS], F32, isOutput=False)
    wi_xy = nc.declare_dram_parameter("wi_xy", [128, NT * 256], F32, isOutput=False)
    w0p = nc.declare_dram_parameter("w0p", [128, 128], F32, isOutput=False)
    w1p = nc.declare_dram_parameter("w1p", [128, 128], F32, isOutput=False)
    w2p = nc.declare_dram_parameter("w2p", [128, 128], F32, isOutput=False)
    w3p = nc.declare_dram_parameter("w3p", [128, 128], F32, isOutput=False)
    b012 = nc.declare_dram_parameter("b012", [128, 3], F32, isOutput=False)
    b3p = nc.declare_dram_parameter("b3p", [128, 1], F32, isOutput=False)
    dcolf = nc.declare_dram_parameter("dcolf", [128, D], F32, isOutput=False)
    ident = nc.declare_dram_parameter("ident", [128, 128], F32, isOutput=False)
    probe = nc.declare_dram_parameter("probe", [128, 8], F32, isOutput=False)
    pdf_fac = nc.declare_dram_parameter("pdf_fac", [ANG * D * RES * RES, 2], F32,
                                        isOutput=False)
    pdf2db = nc.declare_dram_parameter("pdf2db", [ANG * D * 128, RES * RES // 128],
                                       BF16, isOutput=False)
    out_ext = nc.declare_dram_parameter("out", [128, NCHUNK], F32, isOutput=True)
    dbg_ext = nc.declare_dram_parameter("dbg", [128, 16], F32, isOutput=True)
    dbg2_ext = nc.declare_dram_parameter("dbg2", [128, 1280], F32, isOutput=True)

    with tile.TileContext(nc) as tc:
        with (
            tc.tile_pool(name="const", bufs=1) as cpool,
            tc.tile_pool(name="work", bufs=2) as wpool,
            tc.tile_pool(name="math", bufs=1) as mpool,
            tc.tile_pool(name="psum", bufs=2, space="PSUM") as ppool,
            tc.tile_pool(name="psum2", bufs=2, space="PSUM") as ppool2,
        ):
            # ---- constants to SBUF ----
            w0t = cpool.tile([128, 128], F32); nc.sync.dma_start(w0t[:], w0p[:])
            w1t = cpool.tile([128, 128], F32); nc.sync.dma_start(w1t[:], w1p[:])
            w2t = cpool.tile([128, 128], F32); nc.sync.dma_start(w2t[:], w2p[:])
            w3t = cpool.tile([128, 128], F32); nc.sync.dma_start(w3t[:], w3p[:])
            bt = cpool.tile([128, 3], F32); nc.sync.dma_start(bt[:], b012[:])
            b3t = cpool.tile([128, 1], F32); nc.sync.dma_start(b3t[:], b3p[:])
            dct = cpool.tile([128, D], F32); nc.sync.dma_start(dct[:], dcolf[:])
            idt = cpool.tile([128, 128], F32); nc.sync.dma_start(idt[:], ident[:])
            prt = cpool.tile([128, 8], F32); nc.sync.dma_start(prt[:], probe[:])
            rhs = cpool.tile([128, 512], F32)
            nc.vector.memset(rhs[:], 0.0)
            eps24 = cpool.tile([128, 1], F32)
            nc.vector.memset(eps24[:], 1e-24)
            c256 = cpool.tile([128, 1], F32)
            nc.vector.memset(c256[:], 256.0)

            # ---- probe: cvt + mod semantics -> dbg ----
            dbg = cpool.tile([128, 16], F32)
            pri = cpool.tile([128, 8], I32)
            nc.vector.tensor_copy(pri[:], prt[:])
            nc.vector.tensor_copy(dbg[:, 0:8], pri[:])
            nc.vector.memset(dbg[:, 8:16], -777.0)
            nc.sync.dma_start(dbg_ext[:], dbg[:])

            # ---- per-texture pdf sums on PE (bf16 loads) -> norm row ----
            ones = cpool.tile([128, 1], BF16)
            nc.vector.memset(ones[:], 1.0)
            srow = cpool.tile([1, ANG * D], F32)
            for t in range(ANG * D):
                pt = wpool.tile([128, 2048], BF16, tag="pdfsum")
                nc.sync.dma_start(pt[:], pdf2db[t * 128:(t + 1) * 128, :])
                ps = ppool2.tile([1, 512], F32, space="PSUM", tag="pssum")
                for c in range(4):
                    nc.tensor.matmul(ps[:], ones[:], pt[:, c * 512:(c + 1) * 512],
                                     start=(c == 0), stop=(c == 3))
                nc.vector.tensor_reduce(srow[:, t:t + 1], ps[:],
                                        axis=mybir.AxisListType.X, op=AL.add)
            # norm = (RES*RES/4) / max(sum, 1e-12)
            nsr = cpool.tile([1, ANG * D], F32)
            nc.vector.tensor_scalar_max(nsr[:], srow[:], 1e-12)
            nc.vector.reciprocal(nsr[:], nsr[:])
            nc.vector.tensor_scalar_mul(nsr[:], nsr[:], float(RES * RES) / 4.0)
            normt = cpool.tile([128, ANG * D], F32)
            nc.gpsimd.partition_broadcast(normt[:], nsr[:], channels=128)
            # staircase diffs: nd[:, k*8:(k+1)*8] = norm[k] - norm[k-1] (k>=1),
            # nd[:, 0:8] = norm[0]
            nd = cpool.tile([128, ANG * D], F32)
            nc.vector.tensor_copy(nd[:, 0:D], normt[:, 0:D])
            for k in range(1, ANG):
                nc.vector.tensor_tensor(nd[:, k * D:(k + 1) * D],
                                        normt[:, k * D:(k + 1) * D],
                                        normt[:, (k - 1) * D:k * D], op=AL.subtract)

            dbg2 = cpool.tile([128, 1280], F32)
            nc.vector.memset(dbg2[:], 0.0)

            # ---- big state buffers ----
            ot = cpool.tile([128, NCHUNK], F32)
            fi_all = cpool.tile([128, NW], I32)
            pvfac = cpool.tile([128, 2 * NW], F32)
            rl_all = cpool.tile([128, NW], BF16)
            nr_all = cpool.tile([128, NW], F32)
            den_all = cpool.tile([128, NCHUNK], F32)

            for s in range(NT):
                # ---- MLP: block-diagonal weights, one matmul per layer ----
                nc.sync.dma_start(rhs[0:NQ * CIN, :],
                                  cond_t[:, s * 512:(s + 1) * 512])
                h = rhs
                for li, wt_ in enumerate((w0t, w1t, w2t, w3t)):
                    mm = ppool.tile([128, 512], F32, space="PSUM", tag="mm")
                    nc.tensor.matmul(mm[:], wt_[:], h[:], start=True, stop=True)
                    hn = wpool.tile([128, 512], F32, tag=f"h{li % 2}")
                    if li < 3:
                        nc.scalar.activation(hn[:], mm[:], AF.Relu,
                                             bias=bt[:, li:li + 1], scale=1.0)
                    else:
                        nc.scalar.activation(hn[:], mm[:], AF.Identity,
                                             bias=b3t[:, 0:1], scale=1.0)
                    h = hn

                # ---- transpose 4x [128,128] -> chunks ch = c*4+g ----
                tp = ppool.tile([128, 512], F32, space="PSUM", tag="tp")
                for c in range(4):
                    nc.tensor.transpose(
                        tp[:, c * 128:(c + 1) * 128],
                        h[:, c * 128:(c + 1) * 128],
                        idt[:])
                tps = wpool.tile([128, 512], F32, tag="tps")
                nc.scalar.activation(tps[:], tp[:], AF.Copy)

                def blk(base):
                    return tps[:].rearrange("p (ch f) -> p ch f", ch=16)[:, :, base:base + 8]

                WT, VX, VY, ZZ = blk(0), blk(8), blk(16), blk(24)
                cw = s * 128
                wl = wpool.tile([128, 256], F32, tag="wl")
                nc.sync.dma_start(wl[:], wi_xy[:, s * 256:(s + 1) * 256])
                wxs3 = wl[:, 0:128].rearrange("p (ch f) -> p ch f", ch=16)
                wys3 = wl[:, 128:256].rearrange("p (ch f) -> p ch f", ch=16)

                t1 = mpool.tile([128, 16, 8], F32, tag="t1")
                t2 = mpool.tile([128, 16, 8], F32, tag="t2")
                n2 = mpool.tile([128, 16, 8], F32, tag="n2")
                inv = mpool.tile([128, 16, 8], F32, tag="inv")
                nc.scalar.activation(t1[:], VX, AF.Square)
                nc.scalar.activation(t2[:], VY, AF.Square)
                nc.vector.tensor_tensor(n2[:], t1[:], t2[:], op=AL.add)
                nc.scalar.activation(n2[:], n2[:], AF.Sqrt, bias=eps24[:, 0:1])
                nc.vector.reciprocal(inv[:], n2[:])

                rx = mpool.tile([128, 16, 8], F32, tag="rx")
                ry = mpool.tile([128, 16, 8], F32, tag="ry")
                nc.vector.tensor_tensor(t1[:], VX, wxs3, op=AL.mult)
                nc.vector.tensor_tensor(t2[:], VY, wys3, op=AL.mult)
                nc.vector.tensor_tensor(rx[:], t1[:], t2[:], op=AL.subtract)
                nc.vector.tensor_tensor(t1[:], VY, wxs3, op=AL.mult)
                nc.vector.tensor_tensor(t2[:], VX, wys3, op=AL.mult)
                nc.vector.tensor_tensor(ry[:], t1[:], t2[:], op=AL.add)
                nc.vector.tensor_tensor(rx[:], rx[:], inv[:], op=AL.mult)
                nc.vector.tensor_tensor(ry[:], ry[:], inv[:], op=AL.mult)
                # xs = relu(256*r + 256); clip hi at 511.0
                nc.scalar.activation(t1[:], rx[:], AF.Relu, bias=c256[:, 0:1], scale=256.0)
                nc.scalar.activation(t2[:], ry[:], AF.Relu, bias=c256[:, 0:1], scale=256.0)
                nc.vector.tensor_scalar_min(t1[:], t1[:], 511.0)
                nc.vector.tensor_scalar_min(t2[:], t2[:], 511.0)

                xf = mpool.tile([128, 16, 8], F32, tag="xf")
                yf = mpool.tile([128, 16, 8], F32, tag="yf")
                zf = mpool.tile([128, 16, 8], F32, tag="zf")
                ti = mpool.tile([128, 16, 8], I32, tag="ti")
                fmk = mpool.tile([128, 16, 8], F32, tag="fmk")

                def exact_floor(dst_f, src_f):
                    # dst = float(floor(src)) for src >= 0, any cvt rounding
                    nc.vector.tensor_copy(ti[:], src_f)
                    nc.vector.tensor_copy(dst_f[:], ti[:])
                    nc.vector.tensor_tensor(fmk[:], dst_f[:], src_f, op=AL.is_gt)
                    nc.vector.tensor_tensor(dst_f[:], dst_f[:], fmk[:], op=AL.subtract)

                exact_floor(xf, t1[:])
                exact_floor(yf, t2[:])

                # z staircase input: zs = 8*sigmoid(ZZ) + 0.5
                sig = mpool.tile([128, 16, 8], F32, tag="sig")
                zs = mpool.tile([128, 16, 8], F32, tag="zs")
                zsm = mpool.tile([128, 16, 8], F32, tag="zsm")
                nc.scalar.activation(sig[:], ZZ, AF.Sigmoid)
                nc.vector.tensor_scalar(zs[:], sig[:], float(ANG), 0.5,
                                        op0=AL.mult, op1=AL.add)
                nc.vector.tensor_scalar_min(zsm[:], zs[:], 7.9)
                exact_floor(zf, zsm[:])

                # fi = zf*2097152 + d*262144 + yf*512 + xf (exact in f32 < 2^24)
                fi3 = fi_all[:, cw:cw + 128].rearrange("p (ch f) -> p ch f", ch=16)
                dcs = dct[:, 0:D].rearrange("p (o f) -> p o f", o=1).to_broadcast(
                    [128, 16, 8])
                c1 = mpool.tile([128, 16, 8], F32, tag="c1")
                nc.vector.scalar_tensor_tensor(c1[:], zf[:], float(D * RES * RES),
                                               dcs, op0=AL.mult, op1=AL.add)
                nc.vector.scalar_tensor_tensor(c1[:], yf[:], float(RES), c1[:],
                                               op0=AL.mult, op1=AL.add)
                nc.vector.tensor_tensor(c1[:], c1[:], xf[:], op=AL.add)
                nc.vector.tensor_copy(fi3, c1[:])
                if s == 0:
                    nc.vector.tensor_copy(dbg2[:, 0:128],
                                          c1[:].rearrange("p ch f -> p (ch f)"))
                    nc.vector.tensor_copy(dbg2[:, 640:1152], tps[:])

                # norm staircase select: nr = nd0 + sum_k (zs >= k) * nd_k
                nr3 = nr_all[:, cw:cw + 128].rearrange("p (ch f) -> p ch f", ch=16)
                nd0 = nd[:, 0:D].rearrange("p (o f) -> p o f", o=1).to_broadcast(
                    [128, 16, 8])
                tmpm = mpool.tile([128, 16, 8], F32, tag="tmpm")
                nc.vector.tensor_copy(nr3, nd0)
                for k in range(1, ANG):
                    ndk = nd[:, k * D:(k + 1) * D].rearrange(
                        "p (o f) -> p o f", o=1).to_broadcast([128, 16, 8])
                    nc.vector.scalar_tensor_tensor(tmpm[:], zs[:], float(k), ndk,
                                                   op0=AL.is_ge, op1=AL.mult)
                    nc.vector.tensor_tensor(nr3, nr3, tmpm[:], op=AL.add)

                # rl = relu(weight)
                rl3 = rl_all[:, cw:cw + 128].rearrange("p (ch f) -> p ch f", ch=16)
                nc.vector.tensor_scalar_max(rl3, WT, 0.0)

                # ---- gather this supertile's texels (+fac) ----
                # HW indirect DMA consumes one index per partition per
                # instruction: column-wise pair gathers (verified semantics)
                for c in range(128):
                    nc.gpsimd.indirect_dma_start(
                        out=pvfac[:, 2 * (cw + c):2 * (cw + c) + 2],
                        out_offset=None, in_=pdf_fac[:],
                        in_offset=bass.IndirectOffsetOnAxis(
                            ap=fi_all[:, cw + c:cw + c + 1], axis=0))



            # ---- debug captures (supertile 0) ----
            nc.vector.tensor_copy(dbg2[:, 128:384], pvfac[:, 0:256])
            nc.vector.tensor_copy(dbg2[:, 384:512], nr_all[:, 0:128])
            nc.vector.tensor_copy(dbg2[:, 512:640], rl_all[:, 0:128])

            # ---- phase 3: combine ----
            HW2 = NW // 4
            tmp = cpool.tile([128, HW2], F32)
            for hh in range(4):
                cl, cr = hh * HW2, (hh + 1) * HW2
                pv3 = pvfac[:, 2 * cl:2 * cr].rearrange(
                    "p (n two) -> p n two", two=2)[:, :, 0:1]
                fc3 = pvfac[:, 2 * cl:2 * cr].rearrange(
                    "p (n two) -> p n two", two=2)[:, :, 1:2]
                rl3 = rl_all[:, cl:cr].rearrange("p (n o) -> p n o", o=1)
                nr3 = nr_all[:, cl:cr].rearrange("p (n o) -> p n o", o=1)
                t3 = tmp[:].rearrange("p (n o) -> p n o", o=1)
                nc.vector.tensor_tensor(t3, rl3, fc3, op=AL.mult)
                den3 = den_all[:, hh * (NCHUNK // 4):(hh + 1) * (NCHUNK // 4)] \
                    .rearrange("p (c o) -> p c o", o=1)
                nc.vector.tensor_reduce(
                    den3, tmp[:].rearrange("p (c f) -> p c f", f=D),
                    axis=mybir.AxisListType.X, op=AL.add,
                    apply_absolute_value=True)
                nc.vector.tensor_tensor(t3, t3, nr3, op=AL.mult)
                nc.vector.tensor_tensor(t3, t3, pv3, op=AL.mult)
                oth = ot[:, hh * (NCHUNK // 4):(hh + 1) * (NCHUNK // 4)] \
                    .rearrange("p (c o) -> p c o", o=1)
                nc.vector.tensor_reduce(
                    oth, tmp[:].rearrange("p (c f) -> p c f", f=D),
                    axis=mybir.AxisListType.X, op=AL.add)
            nc.vector.tensor_scalar_max(den_all[:], den_all[:], 1e-12)
            nc.vector.reciprocal(den_all[:], den_all[:])
            nc.vector.tensor_tensor(ot[:], ot[:], den_all[:], op=AL.mult)
            dn = min(128, NCHUNK)
            nc.vector.tensor_copy(dbg2[:, 1152:1152 + dn], den_all[:, 0:dn])
            nc.sync.dma_start(out_ext[:], ot[:])
            nc.sync.dma_start(dbg2_ext[:], dbg2[:])
    return nc


def prep_inputs(wi, cond, w0, b0, w1, b1, w2, b2, w3, b3, pdf, fac):
    """Host-side sharding + layout. Returns in_maps (list of 8 dicts)."""
    perm = np.concatenate([np.arange(D),                 # weight
                           D + 2 * np.arange(D),         # vx
                           D + 2 * np.arange(D) + 1,     # vy
                           3 * D + np.arange(D)])        # z
    w3r = w3[:, perm].astype(np.float32)
    b3r = b3[perm].astype(np.float32)

    def packw(w, kk, rstep):
        t = np.zeros((128, 128), np.float32)
        for g in range(NQ):
            t[rstep * g:rstep * g + kk, 32 * g:32 * g + 32] = w
        return t

    w0p = packw(w0, CIN, CIN)          # input rows packed 0..39
    w1p = packw(w1, HID, HID)
    w2p = packw(w2, HID, HID)
    w3p = packw(w3r, HID, HID)
    b012 = np.zeros((128, 3), np.float32)
    b3p = np.zeros((128, 1), np.float32)
    for g in range(NQ):
        for li, b in enumerate((b0, b1, b2)):
            b012[32 * g:32 * g + 32, li] = b
        b3p[32 * g:32 * g + 32, 0] = b3r
    dcolf = np.broadcast_to(
        (np.arange(D, dtype=np.float32) * (RES * RES)).reshape(1, D),
        (128, D)).copy()
    ident = np.eye(128, dtype=np.float32)
    probe = np.broadcast_to(np.array(
        [0.5, 1.5, 2.5, 3.5, 0.49, 0.51, 5.75, 511.0],
        np.float32).reshape(1, 8), (128, 8)).copy()

    # interleaved (pdf, fac) gather table: pure layout, replicated per core
    pdf_fac = np.empty((ANG, D, RES, RES, 2), np.float32)
    pdf_fac[..., 0] = pdf
    pdf_fac[..., 1] = fac[:, :, None, None]
    pdf_fac = pdf_fac.reshape(ANG * D * RES * RES, 2)

    def bf16(x):
        u = np.asarray(x, np.float32).view(np.uint32)
        r = ((u + 0x7FFF + ((u >> 16) & 1)) >> 16).astype(np.uint16)
        return r.view(np.dtype('uint16'))

    import ml_dtypes
    pdf2db = pdf.reshape(ANG * D * 128, RES * RES // 128).astype(ml_dtypes.bfloat16)

    in_maps = []
    for c in range(NC_N):
        sl = slice(c * BC, (c + 1) * BC)
        cond_c = cond[sl].reshape(NQ, QS, CIN)
        cond_t = np.ascontiguousarray(
            cond_c.transpose(0, 2, 1).reshape(NQ * CIN, QS))
        # wi per chunk layout: chunk (s,g,cc): sample g*QS + s*512 + cc*128 + p
        wi_c = wi[sl]
        g_, s_, cc_, p_ = np.meshgrid(np.arange(NQ), np.arange(NT), np.arange(4),
                                      np.arange(128), indexing='ij')
        samp = (g_ * QS + s_ * 512 + cc_ * 128 + p_)
        chunk = (s_ * 16 + cc_ * 4 + g_)
        wx = np.zeros((128, NCHUNK, D), np.float32)
        wy = np.zeros((128, NCHUNK, D), np.float32)
        wx[p_.ravel(), chunk.ravel()] = wi_c[samp.ravel(), 0:1]
        wy[p_.ravel(), chunk.ravel()] = wi_c[samp.ravel(), 1:2]
        wx = wx.reshape(128, NT, 128)
        wy = wy.reshape(128, NT, 128)
        wi_xy = np.concatenate([wx, wy], axis=2).reshape(128, NT * 256)
        in_maps.append(dict(
            cond_t=cond_t, wi_xy=np.ascontiguousarray(wi_xy),
            w0p=w0p, w1p=w1p, w2p=w2p, w3p=w3p, b012=b012, b3p=b3p,
            dcolf=dcolf, ident=ident, probe=probe,
            pdf_fac=pdf_fac, pdf2db=pdf2db))
    return in_maps


def unshard_output(results):
    out = np.empty(B, np.float32)
    g_, s_, cc_, p_ = np.meshgrid(np.arange(NQ), np.arange(NT), np.arange(4),
                                  np.arange(128), indexing='ij')
    samp = (g_ * QS + s_ * 512 + cc_ * 128 + p_).ravel()
    chunk = (s_ * 16 + cc_ * 4 + g_).ravel()
    for c in range(NC_N):
        o = results[c]["out"]  # [128, NCHUNK]
        out[c * BC + samp] = o[p_.ravel(), chunk]
    return out


_CACHE = {}


def kernel(**inputs):
    if 'nc' not in _CACHE:
        _CACHE['nc'] = build_kernel()
    nc = _CACHE['nc']
    if not nc.is_finalized():
        nc.finalize()
    in_maps = prep_inputs(**{k: np.asarray(v) for k, v in inputs.items()})
    r = run_bass_kernel_spmd(nc, in_maps, list(range(NC_N)),
                             trace=bool(os.environ.get("KTRACE")))
    if r.exec_time_ns:
        print(f"HW exec time: {r.exec_time_ns} ns")
    if os.environ.get("KTRACE") and r.instructions_and_trace:
        try:
            import pickle
            insts, tracep = r.instructions_and_trace
            rows = [(i.name, str(i.engine), i.timestamp, i.duration,
                     getattr(i, 'op_name', ''), getattr(i, 'label', ''))
                    for i in insts]
            with open('/tmp/ktrace_insts.pkl', 'wb') as f:
                pickle.dump(rows, f)
            print(f"trace: {tracep} profile_json: {r.profile_json} "
                  f"n_insts: {len(rows)}")
        except Exception as e:
            print("trace dump failed:", e)
    if os.environ.get("KDBG"):
        try:
            d = r.results[0]["dbg"]
            print("probe in :", [0.5, 1.5, 2.5, 3.5, 0.49, 0.51, 5.75, 511.0])
            print("probe cvt:", d[0, 0:8].tolist())
            np.save('/tmp/kdbg2.npy', r.results[0]["dbg2"])
        except Exception as e:
            print("probe failed:", e)
    return unshard_output(r.results)


if __name__ == "__main__":
    pass



# revision 1
# speedup vs baseline: 4.3728x; 4.3728x over previous
"""Trainium2 Bass kernel for nn_Base2DInference (sampling).

Data-parallel over the sample batch B across 8 NeuronCores. Per core:
  - tiny MLP 10->32->32->32->32 in fp32 on the PE, 4 sample-groups packed
    into the 128x128 array (K,M=32 blocks)
  - PE transpose of the MLP head to samples-on-partitions layout
  - index math (rotation, texel indices, z bin) on DVE/ACT
  - per-bin norm factors via an is_ge staircase select
  - one big indirect-DMA gather per supertile from an (pdf,fac)-interleaved
    replicated DRAM table (8B per sample-component)
  - weighted mixture reduction -> [B/8] output
"""
import sys, os, time, types

sys.path.insert(0, '/opt/trn_rl_repo')

import numpy as np


def _install_ntff_hook_shim():
    if 'antenv.axon_hooks' in sys.modules:
        return
    try:
        from trn_agent_boot.trn_boot import _ntff_profile_via_ctypes
        hook = _ntff_profile_via_ctypes('/opt/axon/libaxon_pjrt.so')
    except Exception:
        hook = None
    mod = types.ModuleType('antenv.axon_hooks')
    _state = {'hook': hook}
    mod.set_axon_ntff_profile_hook = lambda h: _state.__setitem__('hook', h)
    mod.get_axon_ntff_profile_hook = lambda: _state['hook']
    sys.modules['antenv.axon_hooks'] = mod


_install_ntff_hook_shim()

from concourse import bass, mybir, bacc, tile
from concourse.bass_utils import run_bass_kernel_spmd

F32 = mybir.dt.float32
BF16 = mybir.dt.bfloat16
I32 = mybir.dt.int32

RES, ANG, D, HID, CIN = 512, 8, 8, 32, 10
B = 1048576
NC_N = 8
BC = B // NC_N            # samples per core: 131072
NQ = 4                    # packed sample groups per matmul
QS = BC // NQ             # samples per group: 32768
NT = QS // 512            # super-tiles: 64
NCHUNK = BC // 128        # 128-sample chunks: 1024
NW = NCHUNK * D           # texel columns: 8192

_MAXW = 1


def _patched_drain_and_barrier(self, tick_clock, wait_clock):
    from concourse.vector_clock import ScopedClock
    nc = self.nc
    drain_inst = nc.sync.drain()
    wait_clock.add_sem_waits(drain_inst.ins, ScopedClock({None: tick_clock.global_clock}))
    si = drain_inst.ins.sync_info
    if si is not None and si.on_wait and len(si.on_wait) > _MAXW:
        waits = list(si.on_wait)
        drain_inst.ins.sync_info = mybir.SyncInfo(
            on_wait=waits[:_MAXW], on_update=list(si.on_update))
        for i in range(_MAXW, len(waits), _MAXW):
            nop = nc.sync.nop(nofuse=True)
            nop.ins.sync_info = mybir.SyncInfo(on_wait=waits[i:i + _MAXW], on_update=[])
    nc.all_engine_barrier()
    popped = nc._tile_sem_poison_stack.pop()
    assert popped is self._sem_poison
    nc.clear_and_free_semaphores(list(self.sems.allocated().values()))
    nc.all_engine_barrier()


tile.TileContext._drain_and_barrier = _patched_drain_and_barrier


def split_sync_waits(nc, max_waits=_MAXW):
    fn = nc.m.functions[0]
    root_bb = nc.cur_bb.bb
    for bb in list(fn.blocks):
        insts = bb.instructions
        changed = False
        out = []
        for inst in insts:
            si = inst.sync_info
            waits = list(si.on_wait) if (si is not None and si.on_wait) else []
            if len(waits) > max_waits:
                changed = True
                extra = waits[:-max_waits]
                inst.sync_info = mybir.SyncInfo(
                    on_wait=waits[-max_waits:], on_update=list(si.on_update))
                for j in range(0, len(extra), max_waits):
                    nop = nc.engines[inst.engine].nop(nofuse=True)
                    nop.ins.sync_info = mybir.SyncInfo(
                        on_wait=extra[j:j + max_waits], on_update=[])
                    root_insts = root_bb.instructions
                    assert root_insts[-1].name == nop.ins.name
                    root_bb.instructions = root_insts[:-1]
                    out.append(nop.ins)
            out.append(inst)
        if changed:
            bb.instructions = out


class PatchedBacc(bacc.Bacc):
    def finalize(self):
        self.compile()
        split_sync_waits(self)
        self.verify_switch_hints()
        self.assert_all_executable()
        self.freeze()
        self._finalized = True


def build_kernel():
    AL = mybir.AluOpType
    AF = mybir.ActivationFunctionType
    nc = PatchedBacc()

    cond_t = nc.declare_dram_parameter("cond_t", [NQ * CIN, QS], F32, isOutput=False)
    wi_xy = nc.declare_dram_parameter("wi_xy", [128, NT * 256], F32, isOutput=False)
    w0p = nc.declare_dram_parameter("w0p", [128, 128], F32, isOutput=False)
    w1p = nc.declare_dram_parameter("w1p", [128, 128], F32, isOutput=False)
    w2p = nc.declare_dram_parameter("w2p", [128, 128], F32, isOutput=False)
    w3p = nc.declare_dram_parameter("w3p", [128, 128], F32, isOutput=False)
    b012 = nc.declare_dram_parameter("b012", [128, 3], F32, isOutput=False)
    b3p = nc.declare_dram_parameter("b3p", [128, 1], F32, isOutput=False)
    dcolf = nc.declare_dram_parameter("dcolf", [128, D], F32, isOutput=False)
    ident = nc.declare_dram_parameter("ident", [128, 128], F32, isOutput=False)
    probe = nc.declare_dram_parameter("probe", [128, 8], F32, isOutput=False)
    pdf_fac = nc.declare_dram_parameter("pdf_fac", [ANG * D * RES * RES, 2], F32,
                                        isOutput=False)
    pdf2db = nc.declare_dram_parameter("pdf2db", [ANG * D * 128, RES * RES // 128],
                                       BF16, isOutput=False)
    out_ext = nc.declare_dram_parameter("out", [128, NCHUNK], F32, isOutput=True)
    dbg_ext = nc.declare_dram_parameter("dbg", [128, 16], F32, isOutput=True)
    dbg2_ext = nc.declare_dram_parameter("dbg2", [128, 1280], F32, isOutput=True)

    with tile.TileContext(nc) as tc:
        with (
            tc.tile_pool(name="const", bufs=1) as cpool,
            tc.tile_pool(name="work", bufs=2) as wpool,
            tc.tile_pool(name="math", bufs=1) as mpool,
            tc.tile_pool(name="psum", bufs=2, space="PSUM") as ppool,
            tc.tile_pool(name="psum2", bufs=2, space="PSUM") as ppool2,
        ):
            # ---- constants to SBUF ----
            w0t = cpool.tile([128, 128], F32); nc.sync.dma_start(w0t[:], w0p[:])
            w1t = cpool.tile([128, 128], F32); nc.sync.dma_start(w1t[:], w1p[:])
            w2t = cpool.tile([128, 128], F32); nc.sync.dma_start(w2t[:], w2p[:])
            w3t = cpool.tile([128, 128], F32); nc.sync.dma_start(w3t[:], w3p[:])
            bt = cpool.tile([128, 3], F32); nc.sync.dma_start(bt[:], b012[:])
            b3t = cpool.tile([128, 1], F32); nc.sync.dma_start(b3t[:], b3p[:])
            dct = cpool.tile([128, D], F32); nc.sync.dma_start(dct[:], dcolf[:])
            idt = cpool.tile([128, 128], F32); nc.sync.dma_start(idt[:], ident[:])
            prt = cpool.tile([128, 8], F32); nc.sync.dma_start(prt[:], probe[:])
            rhs = cpool.tile([128, 512], F32)
            nc.vector.memset(rhs[:], 0.0)
            eps24 = cpool.tile([128, 1], F32)
            nc.vector.memset(eps24[:], 1e-24)
            c256 = cpool.tile([128, 1], F32)
            nc.vector.memset(c256[:], 256.0)

            # ---- probe: cvt + mod semantics -> dbg ----
            dbg = cpool.tile([128, 16], F32)
            pri = cpool.tile([128, 8], I32)
            nc.vector.tensor_copy(pri[:], prt[:])
            nc.vector.tensor_copy(dbg[:, 0:8], pri[:])
            nc.vector.memset(dbg[:, 8:16], -777.0)
            nc.sync.dma_start(dbg_ext[:], dbg[:])

            # ---- per-texture pdf sums on PE (bf16 loads) -> norm row ----
            ones = cpool.tile([128, 1], BF16)
            nc.vector.memset(ones[:], 1.0)
            srow = cpool.tile([1, ANG * D], F32)
            for t in range(ANG * D):
                pt = wpool.tile([128, 2048], BF16, tag="pdfsum")
                nc.sync.dma_start(pt[:], pdf2db[t * 128:(t + 1) * 128, :])
                ps = ppool2.tile([1, 512], F32, space="PSUM", tag="pssum")
                for c in range(4):
                    nc.tensor.matmul(ps[:], ones[:], pt[:, c * 512:(c + 1) * 512],
                                     start=(c == 0), stop=(c == 3))
                nc.vector.tensor_reduce(srow[:, t:t + 1], ps[:],
                                        axis=mybir.AxisListType.X, op=AL.add)
            # norm = (RES*RES/4) / max(sum, 1e-12)
            nsr = cpool.tile([1, ANG * D], F32)
            nc.vector.tensor_scalar_max(nsr[:], srow[:], 1e-12)
            nc.vector.reciprocal(nsr[:], nsr[:])
            nc.vector.tensor_scalar_mul(nsr[:], nsr[:], float(RES * RES) / 4.0)
            normt = cpool.tile([128, ANG * D], F32)
            nc.gpsimd.partition_broadcast(normt[:], nsr[:], channels=128)
            # staircase diffs: nd[:, k*8:(k+1)*8] = norm[k] - norm[k-1] (k>=1),
            # nd[:, 0:8] = norm[0]
            nd = cpool.tile([128, ANG * D], F32)
            nc.vector.tensor_copy(nd[:, 0:D], normt[:, 0:D])
            for k in range(1, ANG):
                nc.vector.tensor_tensor(nd[:, k * D:(k + 1) * D],
                                        normt[:, k * D:(k + 1) * D],
                                        normt[:, (k - 1) * D:k * D], op=AL.subtract)

            dbg2 = cpool.tile([128, 1280], F32)
            nc.vector.memset(dbg2[:], 0.0)

            # ---- big state buffers ----
            ot = cpool.tile([128, NCHUNK], F32)
            fi_all = cpool.tile([128, NW], I32)
            pvfac = cpool.tile([128, 2 * NW], F32)
            rl_all = cpool.tile([128, NW], BF16)
            nr_all = cpool.tile([128, NW], F32)
            den_all = cpool.tile([128, NCHUNK], F32)

            for s in range(NT):
                # ---- MLP: block-diagonal weights, one matmul per layer ----
                nc.sync.dma_start(rhs[0:NQ * CIN, :],
                                  cond_t[:, s * 512:(s + 1) * 512])
                h = rhs
                for li, wt_ in enumerate((w0t, w1t, w2t, w3t)):
                    mm = ppool.tile([128, 512], F32, space="PSUM", tag="mm")
                    nc.tensor.matmul(mm[:], wt_[:], h[:], start=True, stop=True)
                    hn = wpool.tile([128, 512], F32, tag=f"h{li % 2}")
                    if li < 3:
                        nc.scalar.activation(hn[:], mm[:], AF.Relu,
                                             bias=bt[:, li:li + 1], scale=1.0)
                    else:
                        nc.scalar.activation(hn[:], mm[:], AF.Identity,
                                             bias=b3t[:, 0:1], scale=1.0)
                    h = hn

                # ---- transpose 4x [128,128] -> chunks ch = c*4+g ----
                tp = ppool.tile([128, 512], F32, space="PSUM", tag="tp")
                for c in range(4):
                    nc.tensor.transpose(
                        tp[:, c * 128:(c + 1) * 128],
                        h[:, c * 128:(c + 1) * 128],
                        idt[:])
                tps = wpool.tile([128, 512], F32, tag="tps")
                nc.scalar.activation(tps[:], tp[:], AF.Copy)

                def blk(base):
                    return tps[:].rearrange("p (ch f) -> p ch f", ch=16)[:, :, base:base + 8]

                WT, VX, VY, ZZ = blk(0), blk(8), blk(16), blk(24)
                cw = s * 128
                wl = wpool.tile([128, 256], F32, tag="wl")
                nc.sync.dma_start(wl[:], wi_xy[:, s * 256:(s + 1) * 256])
                wxs3 = wl[:, 0:128].rearrange("p (ch f) -> p ch f", ch=16)
                wys3 = wl[:, 128:256].rearrange("p (ch f) -> p ch f", ch=16)

                t1 = mpool.tile([128, 16, 8], F32, tag="t1")
                t2 = mpool.tile([128, 16, 8], F32, tag="t2")
                n2 = mpool.tile([128, 16, 8], F32, tag="n2")
                inv = mpool.tile([128, 16, 8], F32, tag="inv")
                nc.scalar.activation(t1[:], VX, AF.Square)
                nc.scalar.activation(t2[:], VY, AF.Square)
                nc.vector.tensor_tensor(n2[:], t1[:], t2[:], op=AL.add)
                nc.scalar.activation(n2[:], n2[:], AF.Sqrt, bias=eps24[:, 0:1])
                nc.vector.reciprocal(inv[:], n2[:])

                rx = mpool.tile([128, 16, 8], F32, tag="rx")
                ry = mpool.tile([128, 16, 8], F32, tag="ry")
                nc.vector.tensor_tensor(t1[:], VX, wxs3, op=AL.mult)
                nc.vector.tensor_tensor(t2[:], VY, wys3, op=AL.mult)
                nc.vector.tensor_tensor(rx[:], t1[:], t2[:], op=AL.subtract)
                nc.vector.tensor_tensor(t1[:], VY, wxs3, op=AL.mult)
                nc.vector.tensor_tensor(t2[:], VX, wys3, op=AL.mult)
                nc.vector.tensor_tensor(ry[:], t1[:], t2[:], op=AL.add)
                nc.vector.tensor_tensor(rx[:], rx[:], inv[:], op=AL.mult)
                nc.vector.tensor_tensor(ry[:], ry[:], inv[:], op=AL.mult)
                # xs = relu(256*r + 256); clip hi at 511.0
                nc.scalar.activation(t1[:], rx[:], AF.Relu, bias=c256[:, 0:1], scale=256.0)
                nc.scalar.activation(t2[:], ry[:], AF.Relu, bias=c256[:, 0:1], scale=256.0)
                nc.vector.tensor_scalar_min(t1[:], t1[:], 511.0)
                nc.vector.tensor_scalar_min(t2[:], t2[:], 511.0)

                xf = mpool.tile([128, 16, 8], F32, tag="xf")
                yf = mpool.tile([128, 16, 8], F32, tag="yf")
                zf = mpool.tile([128, 16, 8], F32, tag="zf")
                ti = mpool.tile([128, 16, 8], I32, tag="ti")
                fmk = mpool.tile([128, 16, 8], F32, tag="fmk")

                def exact_floor(dst_f, src_f):
                    # dst = float(floor(src)) for src >= 0, any cvt rounding
                    nc.vector.tensor_copy(ti[:], src_f)
                    nc.vector.tensor_copy(dst_f[:], ti[:])
                    nc.vector.tensor_tensor(fmk[:], dst_f[:], src_f, op=AL.is_gt)
                    nc.vector.tensor_tensor(dst_f[:], dst_f[:], fmk[:], op=AL.subtract)

                exact_floor(xf, t1[:])
                exact_floor(yf, t2[:])

                # z staircase input: zs = 8*sigmoid(ZZ) + 0.5
                sig = mpool.tile([128, 16, 8], F32, tag="sig")
                zs = mpool.tile([128, 16, 8], F32, tag="zs")
                zsm = mpool.tile([128, 16, 8], F32, tag="zsm")
                nc.scalar.activation(sig[:], ZZ, AF.Sigmoid)
                nc.vector.tensor_scalar(zs[:], sig[:], float(ANG), 0.5,
                                        op0=AL.mult, op1=AL.add)
                nc.vector.tensor_scalar_min(zsm[:], zs[:], 7.9)
                exact_floor(zf, zsm[:])

                # fi = zf*2097152 + d*262144 + yf*512 + xf (exact in f32 < 2^24)
                fi3 = fi_all[:, cw:cw + 128].rearrange("p (ch f) -> p ch f", ch=16)
                dcs = dct[:, 0:D].rearrange("p (o f) -> p o f", o=1).to_broadcast(
                    [128, 16, 8])
                c1 = mpool.tile([128, 16, 8], F32, tag="c1")
                nc.vector.scalar_tensor_tensor(c1[:], zf[:], float(D * RES * RES),
                                               dcs, op0=AL.mult, op1=AL.add)
                nc.vector.scalar_tensor_tensor(c1[:], yf[:], float(RES), c1[:],
                                               op0=AL.mult, op1=AL.add)
                nc.vector.tensor_tensor(c1[:], c1[:], xf[:], op=AL.add)
                nc.vector.tensor_copy(fi3, c1[:])
                if s == 0:
                    nc.vector.tensor_copy(dbg2[:, 0:128],
                                          c1[:].rearrange("p ch f -> p (ch f)"))
                    nc.vector.tensor_copy(dbg2[:, 640:1152], tps[:])

                # norm staircase select: nr = nd0 + sum_k (zs >= k) * nd_k
                nr3 = nr_all[:, cw:cw + 128].rearrange("p (ch f) -> p ch f", ch=16)
                nd0 = nd[:, 0:D].rearrange("p (o f) -> p o f", o=1).to_broadcast(
                    [128, 16, 8])
                tmpm = mpool.tile([128, 16, 8], F32, tag="tmpm")
                nc.vector.tensor_copy(nr3, nd0)
                for k in range(1, ANG):
                    ndk = nd[:, k * D:(k + 1) * D].rearrange(
                        "p (o f) -> p o f", o=1).to_broadcast([128, 16, 8])
                    nc.vector.scalar_tensor_tensor(tmpm[:], zs[:], float(k), ndk,
                                                   op0=AL.is_ge, op1=AL.mult)
                    nc.vector.tensor_tensor(nr3, nr3, tmpm[:], op=AL.add)

                # rl = relu(weight)
                rl3 = rl_all[:, cw:cw + 128].rearrange("p (ch f) -> p ch f", ch=16)
                nc.vector.tensor_scalar_max(rl3, WT, 0.0)

                # ---- gather this supertile's texels (+fac) ----
                # HW indirect DMA consumes one index per partition per
                # instruction: column-wise pair gathers (verified semantics)
                for c in range(128):
                    nc.gpsimd.indirect_dma_start(
                        out=pvfac[:, 2 * (cw + c):2 * (cw + c) + 2],
                        out_offset=None, in_=pdf_fac[:],
                        in_offset=bass.IndirectOffsetOnAxis(
                            ap=fi_all[:, cw + c:cw + c + 1], axis=0))



            # ---- debug captures (supertile 0) ----
            nc.vector.tensor_copy(dbg2[:, 128:384], pvfac[:, 0:256])
            nc.vector.tensor_copy(dbg2[:, 384:512], nr_all[:, 0:128])
            nc.vector.tensor_copy(dbg2[:, 512:640], rl_all[:, 0:128])

            # ---- phase 3: combine ----
            HW2 = NW // 4
            tmp = cpool.tile([128, HW2], F32)
            for hh in range(4):
                cl, cr = hh * HW2, (hh + 1) * HW2
                pv3 = pvfac[:, 2 * cl:2 * cr].rearrange(
                    "p (n two) -> p n two", two=2)[:, :, 0:1]
                fc3 = pvfac[:, 2 * cl:2 * cr].rearrange(
                    "p (n two) -> p n two", two=2)[:, :, 1:2]
                rl3 = rl_all[:, cl:cr].rearrange("p (n o) -> p n o", o=1)
                nr3 = nr_all[:, cl:cr].rearrange("p (n o) -> p n o", o=1)
                t3 = tmp[:].rearrange("p (n o) -> p n o", o=1)
                nc.vector.tensor_tensor(t3, rl3, fc3, op=AL.mult)
                den3 = den_all[:, hh * (NCHUNK // 4):(hh + 1) * (NCHUNK // 4)] \
                    .rearrange("p (c o) -> p c o", o=1)
                nc.vector.tensor_reduce(
                    den3, tmp[:].rearrange("p (c f) -> p c f", f=D),
                    axis=mybir.AxisListType.X, op=AL.add,
                    apply_absolute_value=True)
                nc.vector.tensor_tensor(t3, t3, nr3, op=AL.mult)
                nc.vector.tensor_tensor(t3, t3, pv3, op=AL.mult)
                oth = ot[:, hh * (NCHUNK // 4):(hh + 1) * (NCHUNK // 4)] \
                    .rearrange("p (c o) -> p c o", o=1)
                nc.vector.tensor_reduce(
                    oth, tmp[:].rearrange("p (c f) -> p c f", f=D),
                    axis=mybir.AxisListType.X, op=AL.add)
            nc.vector.tensor_scalar_max(den_all[:], den_all[:], 1e-12)
            nc.vector.reciprocal(den_all[:], den_all[:])
            nc.vector.tensor_tensor(ot[:], ot[:], den_all[:], op=AL.mult)
            dn = min(128, NCHUNK)
            nc.vector.tensor_copy(dbg2[:, 1152:1152 + dn], den_all[:, 0:dn])
            nc.sync.dma_start(out_ext[:], ot[:])
            nc.sync.dma_start(dbg2_ext[:], dbg2[:])
    return nc


def prep_inputs(wi, cond, w0, b0, w1, b1, w2, b2, w3, b3, pdf, fac):
    """Host-side sharding + layout. Returns in_maps (list of 8 dicts)."""
    perm = np.concatenate([np.arange(D),                 # weight
                           D + 2 * np.arange(D),         # vx
                           D + 2 * np.arange(D) + 1,     # vy
                           3 * D + np.arange(D)])        # z
    w3r = w3[:, perm].astype(np.float32)
    b3r = b3[perm].astype(np.float32)

    def packw(w, kk, rstep):
        t = np.zeros((128, 128), np.float32)
        for g in range(NQ):
            t[rstep * g:rstep * g + kk, 32 * g:32 * g + 32] = w
        return t

    w0p = packw(w0, CIN, CIN)          # input rows packed 0..39
    w1p = packw(w1, HID, HID)
    w2p = packw(w2, HID, HID)
    w3p = packw(w3r, HID, HID)
    b012 = np.zeros((128, 3), np.float32)
    b3p = np.zeros((128, 1), np.float32)
    for g in range(NQ):
        for li, b in enumerate((b0, b1, b2)):
            b012[32 * g:32 * g + 32, li] = b
        b3p[32 * g:32 * g + 32, 0] = b3r
    dcolf = np.broadcast_to(
        (np.arange(D, dtype=np.float32) * (RES * RES)).reshape(1, D),
        (128, D)).copy()
    ident = np.eye(128, dtype=np.float32)
    probe = np.broadcast_to(np.array(
        [0.5, 1.5, 2.5, 3.5, 0.49, 0.51, 5.75, 511.0],
        np.float32).reshape(1, 8), (128, 8)).copy()

    # interleaved (pdf, fac) gather table: pure layout, replicated per core
    pdf_fac = np.empty((ANG, D, RES, RES, 2), np.float32)
    pdf_fac[..., 0] = pdf
    pdf_fac[..., 1] = fac[:, :, None, None]
    pdf_fac = pdf_fac.reshape(ANG * D * RES * RES, 2)

    def bf16(x):
        u = np.asarray(x, np.float32).view(np.uint32)
        r = ((u + 0x7FFF + ((u >> 16) & 1)) >> 16).astype(np.uint16)
        return r.view(np.dtype('uint16'))

    import ml_dtypes
    pdf2db = pdf.reshape(ANG * D * 128, RES * RES // 128).astype(ml_dtypes.bfloat16)

    in_maps = []
    for c in range(NC_N):
        sl = slice(c * BC, (c + 1) * BC)
        cond_c = cond[sl].reshape(NQ, QS, CIN)
        cond_t = np.ascontiguousarray(
            cond_c.transpose(0, 2, 1).reshape(NQ * CIN, QS))
        # wi per chunk layout: chunk (s,g,cc): sample g*QS + s*512 + cc*128 + p
        wi_c = wi[sl]
        g_, s_, cc_, p_ = np.meshgrid(np.arange(NQ), np.arange(NT), np.arange(4),
                                      np.arange(128), indexing='ij')
        samp = (g_ * QS + s_ * 512 + cc_ * 128 + p_)
        chunk = (s_ * 16 + cc_ * 4 + g_)
        wx = np.zeros((128, NCHUNK, D), np.float32)
        wy = np.zeros((128, NCHUNK, D), np.float32)
        wx[p_.ravel(), chunk.ravel()] = wi_c[samp.ravel(), 0:1]
        wy[p_.ravel(), chunk.ravel()] = wi_c[samp.ravel(), 1:2]
        wx = wx.reshape(128, NT, 128)
        wy = wy.reshape(128, NT, 128)
        wi_xy = np.concatenate([wx, wy], axis=2).reshape(128, NT * 256)
        in_maps.append(dict(
            cond_t=cond_t, wi_xy=np.ascontiguousarray(wi_xy),
            w0p=w0p, w1p=w1p, w2p=w2p, w3p=w3p, b012=b012, b3p=b3p,
            dcolf=dcolf, ident=ident, probe=probe,
            pdf_fac=pdf_fac, pdf2db=pdf2db))
    return in_maps


def unshard_output(results):
    out = np.empty(B, np.float32)
    g_, s_, cc_, p_ = np.meshgrid(np.arange(NQ), np.arange(NT), np.arange(4),
                                  np.arange(128), indexing='ij')
    samp = (g_ * QS + s_ * 512 + cc_ * 128 + p_).ravel()
    chunk = (s_ * 16 + cc_ * 4 + g_).ravel()
    for c in range(NC_N):
        o = results[c]["out"]  # [128, NCHUNK]
        out[c * BC + samp] = o[p_.ravel(), chunk]
    return out


_CACHE = {}


def kernel(**inputs):
    if 'nc' not in _CACHE:
        _CACHE['nc'] = build_kernel()
    nc = _CACHE['nc']
    if not nc.is_finalized():
        nc.finalize()
    in_maps = prep_inputs(**{k: np.asarray(v) for k, v in inputs.items()})
    r = run_bass_kernel_spmd(nc, in_maps, list(range(NC_N)),
                             trace=bool(os.environ.get("KTRACE")))
    if r.exec_time_ns:
        print(f"HW exec time: {r.exec_time_ns} ns")
    if os.environ.get("KTRACE") and r.instructions_and_trace:
        try:
            import pickle
            insts, tracep = r.instructions_and_trace
            rows = [(i.name, str(i.engine), i.timestamp, i.duration,
                     getattr(i, 'op_name', ''), getattr(i, 'label', ''))
                    for i in insts]
            with open('/tmp/ktrace_insts.pkl', 'wb') as f:
                pickle.dump(rows, f)
            print(f"trace: {tracep} profile_json: {r.profile_json} "
                  f"n_insts: {len(rows)}")
        except Exception as e:
            print("trace dump failed:", e)
    if os.environ.get("KDBG"):
        try:
            d = r.results[0]["dbg"]
            print("probe in :", [0.5, 1.5, 2.5, 3.5, 0.49, 0.51, 5.75, 511.0])
            print("probe cvt:", d[0, 0:8].tolist())
            np.save('/tmp/kdbg2.npy', r.results[0]["dbg2"])
        except Exception as e:
            print("probe failed:", e)
    return unshard_output(r.results)


if __name__ == "__main__":
    pass

